# revision 35
# baseline (speedup 1.0000x reference)
"""Trainium2 Bass kernel for AttnNoProjVal.

Per batch element b (one NeuronCore each, B=8), using the identity
  scores = q k^T = hs M hs^T + (hs u) 1^T + 1 (hs v)^T + bk.bq,
  M = Wk^T Wq (host-folded), u = Wk^T bq, v = Wq^T bk:
the v and constant terms are per-QUERY offsets that cancel in softmax and
are dropped; the u term is a per-KEY offset folded into the exp bias.

This version adds KEY COMPACTION: padded keys (mask=True) contribute
exactly zero attention weight, so the host gathers the unmasked keys and
the kernel only computes scores/values over KC = ceil(max_unmasked/128)
key chunks (padding the remainder with bias=-1e30 -> E=0). For the 10%
mask rate this cuts the two big S x S x H matmuls by ~2/16.

Pipeline (all matmul operands fp16 = full PE rate; fp8 was measured on HW
to give 2x only with both operands e4m3, whose 3.6% quantization noise
fails the 2e-2 gate, and residual-split costs 1.5x -> fp16 is optimal):
  A: g^T = M^T hs_c^T           [d, k_compact]   (psum f32 -> fp16 SBUF)
  B: scoresT[k,q] = g^T . hs^T  -> E = exp(s/32 + bias[k]) fp16,
     esum[kp,q] += E (DVE, f32) per key-chunk
  C: colsum[q] = ones^T esum (one N=1 matmul per 128-query tile),
     out[q,:] = (E^T hs_c) * (1/colsum)   (psum f32, DVE scale, DMA out)

The -3 logit shift keeps exp in fp16 range and cancels in the division.
esum replaces the per-chunk N=1 colsum matmuls of the previous version,
whose unhideable weight loads cost ~18us of PE bubbles.
"""

import sys

sys.path.insert(0, "/opt/trn_rl_repo")

from contextlib import ExitStack

import numpy as np

import concourse.tile as tile
from concourse import bacc, mybir
from concourse.bass_utils import run_bass_kernel_spmd

B, S, H = 8, 2048, 1024
N_CORES = 8
HC = H // 128          # 8 chunks of the hidden dim
QB = S // 512          # 4 query blocks for the score matmuls
QS = S // 128          # 16 query tiles for the value matmuls
F32 = mybir.dt.float32
F16 = mybir.dt.float16

_CACHED = {}           # KC -> built Bacc


def _kblocks(kcw):
    """Split the compact key width into <=512-wide moving blocks.

    The first block is kept smaller (384) so its DMA lands right as the PE
    warm-up ends, while staying wide enough that weight loads hide behind
    the moving stream.
    """
    first = min(384, kcw)
    rest = kcw - first
    out = [(0, first)]
    if rest:
        nb = -(-rest // 512)
        base = rest // nb
        base -= base % 32
        widths = [base] * (nb - 1)
        widths.append(rest - base * (nb - 1))
        off = first
        for w in widths:
            out.append((off, w))
            off += w
    return out


def build_nc(kc):
    kcw = kc * 128
    nc = bacc.Bacc(None, target_bir_lowering=False)

    m16 = nc.dram_tensor("m16", [128, HC, H], F16, kind="ExternalInput")
    hstc = nc.dram_tensor("hstc", [128, HC, kcw], F16, kind="ExternalInput")
    hst = nc.dram_tensor("hst", [128, HC, S], F16, kind="ExternalInput")
    hsb = nc.dram_tensor("hsb", [128, kc, H], F16, kind="ExternalInput")
    mkc = nc.dram_tensor("mkc", [128, kc], F32, kind="ExternalInput")
    out = nc.dram_tensor("out", [S, H], F32, kind="ExternalOutput")

    with tile.TileContext(nc) as tc, ExitStack() as whole:
        singles = whole.enter_context(tc.tile_pool(name="singles", bufs=1))
        res_pool = whole.enter_context(tc.tile_pool(name="res", bufs=1))

        junk = singles.tile([128, 512], F16, tag="junk", name="junk")
        nc.vector.memset(junk[:], 0.0)
        bias_sb = singles.tile([128, kc], F32, tag="bias", name="bias_sb")
        ones16 = singles.tile([128, 1], F16, tag="ones", name="ones16")
        nc.vector.memset(ones16[:], 1.0)

        # resident operands
        g16 = res_pool.tile([128, HC, kcw], F16, tag="g16", name="g16")
        hst_sb = res_pool.tile([128, HC, S], F16, tag="hst", name="hst_sb")
        hsb_sb = res_pool.tile([128, kc, H], F16, tag="hsb", name="hsb_sb")

        # PE warm-up: keep the PE ticking through the initial DMA wait.
        with tc.tile_pool(name="psw", bufs=1, space="PSUM") as psw:
            pjunk = psw.tile([128, 512], F32, tag="pj", name="pj")
            for _ in range(12):
                nc.tensor.matmul(
                    pjunk[:], lhsT=junk[:, 0:128], rhs=junk[:], start=True, stop=True
                )

        kbs = _kblocks(kcw)

        # ---- Phase A: fused projection g^T = M^T hs_c^T, fp16 into SBUF.
        # All input DMAs go on one queue, ordered by when the PE needs them:
        # m/hc block 0 first (first chain), then the remaining blocks, then
        # the phase-B/C residents. Serializing avoids bandwidth contention
        # on the startup-critical loads.
        with ExitStack() as pa:
            wt_pool = pa.enter_context(tc.tile_pool(name="wtp", bufs=1))
            hc_pool = pa.enter_context(tc.tile_pool(name="hcp", bufs=1))
            psA = pa.enter_context(tc.tile_pool(name="psA", bufs=4, space="PSUM"))

            # per-dout-chunk m tiles: the tile framework tracks dependencies
            # at tile granularity, so chain oc must only wait for its own
            # 0.25MB slice rather than the whole 2MB of M.
            m_sb = [
                wt_pool.tile([128, HC, 128], F16, tag=f"m{oc}", name=f"m{oc}")
                for oc in range(HC)
            ]
            # first m slice + first key block lead the sync queue (the fast
            # DMA engine) so the first chain's operands land right as the
            # PE warm-up ends. The gpsimd DMA engine is several times
            # slower for these strided patterns — only the tiny bias load
            # goes there.
            # two parallel input streams: M (then the B/C residents) on the
            # sync engine, ALL key blocks upfront on the scalar engine --
            # its queue is empty until phase A's first psum copy retires,
            # so every hc descriptor issues at t~0 and streams concurrently
            # with the m loads instead of serializing behind them.
            nc.sync.dma_start(out=m_sb[0][:], in_=m16.ap()[:, :, 0:128])
            hcs = []
            for bi, (off, bw) in enumerate(kbs):
                t = hc_pool.tile([128, HC, bw], F16, tag=f"hc{bi}", name=f"hc{bi}")
                nc.scalar.dma_start(out=t[:], in_=hstc.ap()[:, :, off:off + bw])
                hcs.append(t)
            nc.gpsimd.dma_start(out=bias_sb[:], in_=mkc.ap())
            for oc in range(1, HC):
                nc.sync.dma_start(
                    out=m_sb[oc][:],
                    in_=m16.ap()[:, :, oc * 128:(oc + 1) * 128],
                )
            for qb in range(QB):
                nc.sync.dma_start(
                    out=hst_sb[:, :, qb * 512:(qb + 1) * 512],
                    in_=hst.ap()[:, :, qb * 512:(qb + 1) * 512],
                )
            nc.sync.dma_start(out=hsb_sb[:], in_=hsb.ap())

            for bi, (off, bw) in enumerate(kbs):
                hc = hcs[bi]
                for oc in range(HC):
                    ps = psA.tile([128, 512], F32, tag="psA", name="psa")
                    for h in range(HC):
                        nc.tensor.matmul(
                            ps[:, 0:bw],
                            lhsT=m_sb[oc][:, h, :],
                            rhs=hc[:, h, :],
                            start=(h == 0),
                            stop=(h == HC - 1),
                        )
                    nc.scalar.copy(out=g16[:, oc, off:off + bw], in_=ps[:, 0:bw])

        # ---- Phase B: scoresT -> exp -> esum, per (key-chunk, query-block).
        with ExitStack() as pb:
            et_pool = pb.enter_context(tc.tile_pool(name="etp", bufs=1))
            es_pool = pb.enter_context(tc.tile_pool(name="esp", bufs=1))

            et = et_pool.tile([128, kc, S], F16, tag="et", name="et")
            esum = es_pool.tile([128, S], F16, tag="esum", name="esum")

            with tc.tile_pool(name="pss", bufs=3, space="PSUM") as ps_s:
                for c in range(kc):
                    for qb in range(QB):
                        ps = ps_s.tile([128, 512], F32, tag="pss", name="pss")
                        for d in range(HC):
                            nc.tensor.matmul(
                                ps[:],
                                lhsT=g16[:, d, c * 128:(c + 1) * 128],
                                rhs=hst_sb[:, d, qb * 512:(qb + 1) * 512],
                                start=(d == 0),
                                stop=(d == HC - 1),
                            )
                        sl = slice(qb * 512, (qb + 1) * 512)
                        nc.scalar.activation(
                            out=et[:, c, sl], in_=ps[:],
                            func=mybir.ActivationFunctionType.Exp,
                            scale=1.0 / 32.0,
                            bias=bias_sb[:, c:c + 1],
                        )
                        if c == 0:
                            nc.vector.tensor_scalar_mul(
                                out=esum[:, sl], in0=et[:, 0, sl], scalar1=1.0
                            )
                        else:
                            nc.vector.scalar_tensor_tensor(
                                out=esum[:, sl], in0=et[:, c, sl], scalar=1.0,
                                in1=esum[:, sl],
                                op0=mybir.AluOpType.mult, op1=mybir.AluOpType.add,
                            )

            # ---- Phase C: attention-value + normalization, per query tile.
            with ExitStack() as pc:
                ps_o = pc.enter_context(tc.tile_pool(name="pso", bufs=3, space="PSUM"))
                ps_n = pc.enter_context(tc.tile_pool(name="psn", bufs=2, space="PSUM"))
                out_pool = pc.enter_context(tc.tile_pool(name="outp", bufs=2))
                r_pool = pc.enter_context(tc.tile_pool(name="rp", bufs=4))

                for qs in range(QS):
                    qsl = slice(qs * 128, (qs + 1) * 128)
                    pn = ps_n.tile([128, 1], F32, tag="pn", name="pn")
                    nc.tensor.matmul(
                        pn[:], lhsT=esum[:, qsl], rhs=ones16[:], start=True, stop=True
                    )
                    r = r_pool.tile([128, 1], F32, tag="r", name="r")
                    nc.vector.reciprocal(r[:], pn[:, 0:1])
                    po0 = ps_o.tile([128, 512], F32, tag="po0", name="po0")
                    po1 = ps_o.tile([128, 512], F32, tag="po1", name="po1")
                    for c in range(kc):
                        st, sp = (c == 0), (c == kc - 1)
                        lw = et[:, c, qsl]
                        nc.tensor.matmul(po0[:], lhsT=lw, rhs=hsb_sb[:, c, 0:512],
                                         start=st, stop=sp)
                        nc.tensor.matmul(po1[:], lhsT=lw, rhs=hsb_sb[:, c, 512:1024],
                                         start=st, stop=sp)
                    ot = out_pool.tile([128, H], F32, tag="ot", name="ot")
                    rows = slice(qs * 128, (qs + 1) * 128)
                    nc.vector.tensor_scalar_mul(out=ot[:, 0:512], in0=po0[:], scalar1=r[:])
                    nc.scalar.dma_start(out=out.ap()[rows, 0:512], in_=ot[:, 0:512])
                    nc.vector.tensor_scalar_mul(out=ot[:, 512:1024], in0=po1[:], scalar1=r[:])
                    nc.scalar.dma_start(out=out.ap()[rows, 512:1024], in_=ot[:, 512:1024])

    nc.finalize()
    return nc


def _pack_T(x):
    """[S', H] -> hs^T arranged [128, HC, S'] (partition = h%128, chunk = h//128)."""
    return np.ascontiguousarray(
        x.T.reshape(HC, 128, x.shape[0]).transpose(1, 0, 2)
    )


def prepare(hidden_states, key_padding_mask, Wq_w, Wq_b, Wk_w, Wk_b):
    """Host-side folding + sharding. Returns (kc, in_maps)."""
    hs = np.ascontiguousarray(hidden_states, dtype=np.float32)
    mask = np.asarray(key_padding_mask, dtype=bool)
    wq = np.asarray(Wq_w, dtype=np.float64)
    wk = np.asarray(Wk_w, dtype=np.float64)
    bq = np.asarray(Wq_b, dtype=np.float64)
    m = (wk.T @ wq).astype(np.float32)                         # [h, d]
    u = (wk.T @ bq).astype(np.float32)                         # [h]

    counts = (~mask).sum(axis=1)
    kc = max(1, min(S // 128, int(-(-counts.max() // 128))))
    kcw = kc * 128

    m16 = np.ascontiguousarray(
        m.reshape(HC, 128, H).transpose(1, 0, 2).astype(np.float16)
    )

    in_maps = []
    for b in range(B):
        idx = np.nonzero(~mask[b])[0]
        hc = np.zeros((kcw, H), np.float32)
        hc[: len(idx)] = hs[b, idx]
        bias = np.full(kcw, -1e30, np.float32)
        bias[: len(idx)] = (hc[: len(idx)] @ u) / 32.0 - 3.0
        bias = np.ascontiguousarray(bias.reshape(kc, 128).T)   # [128, kc]
        h16 = hs[b].astype(np.float16)
        hc16 = hc.astype(np.float16)
        in_maps.append({
            "m16": m16,
            "hstc": _pack_T(hc16),
            "hst": _pack_T(h16),
            "hsb": np.ascontiguousarray(hc16.reshape(kc, 128, H).transpose(1, 0, 2)),
            "mkc": bias,
        })
    return kc, in_maps


def kernel(hidden_states, key_padding_mask, Wq_w, Wq_b, Wk_w, Wk_b):
    kc, in_maps = prepare(hidden_states, key_padding_mask, Wq_w, Wq_b, Wk_w, Wk_b)
    if kc not in _CACHED:
        _CACHED[kc] = build_nc(kc)
    nc = _CACHED[kc]
    res = run_bass_kernel_spmd(nc, in_maps, core_ids=list(range(N_CORES)))
    return np.stack([res.results[b]["out"] for b in range(B)]).astype(np.float32)


# revision 36
# speedup vs baseline: 1.0260x; 1.0260x over previous
"""Trainium2 Bass kernel for AttnNoProjVal.

Per batch element b (one NeuronCore each, B=8), using the identity
  scores = q k^T = hs M hs^T + (hs u) 1^T + 1 (hs v)^T + bk.bq,
  M = Wk^T Wq (host-folded), u = Wk^T bq, v = Wq^T bk:
the v and constant terms are per-QUERY offsets that cancel in softmax and
are dropped; the u term is a per-KEY offset folded into the exp bias.

This version adds KEY COMPACTION: padded keys (mask=True) contribute
exactly zero attention weight, so the host gathers the unmasked keys and
the kernel only computes scores/values over KC = ceil(max_unmasked/128)
key chunks (padding the remainder with bias=-1e30 -> E=0). For the 10%
mask rate this cuts the two big S x S x H matmuls by ~2/16.

Pipeline (all matmul operands fp16 = full PE rate; fp8 was measured on HW
to give 2x only with both operands e4m3, whose 3.6% quantization noise
fails the 2e-2 gate, and residual-split costs 1.5x -> fp16 is optimal):
  A: g^T = M^T hs_c^T           [d, k_compact]   (psum f32 -> fp16 SBUF)
  B: scoresT[k,q] = g^T . hs^T  -> E = exp(s/32 + bias[k]) fp16,
     esum[kp,q] += E (DVE, f32) per key-chunk
  C: colsum[q] = ones^T esum (one N=1 matmul per 128-query tile),
     out[q,:] = (E^T hs_c) * (1/colsum)   (psum f32, DVE scale, DMA out)

The -3 logit shift keeps exp in fp16 range and cancels in the division.
esum replaces the per-chunk N=1 colsum matmuls of the previous version,
whose unhideable weight loads cost ~18us of PE bubbles.
"""

import sys

sys.path.insert(0, "/opt/trn_rl_repo")

from contextlib import ExitStack

import numpy as np

import concourse.tile as tile
from concourse import bacc, mybir
from concourse.bass_utils import run_bass_kernel_spmd

B, S, H = 8, 2048, 1024
N_CORES = 8
HC = H // 128          # 8 chunks of the hidden dim
QB = S // 512          # 4 query blocks for the score matmuls
QS = S // 128          # 16 query tiles for the value matmuls
F32 = mybir.dt.float32
F16 = mybir.dt.float16

_CACHED = {}           # KC -> built Bacc


def _kblocks(kcw):
    """Split the compact key width into <=512-wide moving blocks.

    The first block is kept smaller (384) so its DMA lands right as the PE
    warm-up ends, while staying wide enough that weight loads hide behind
    the moving stream.
    """
    first = min(384, kcw)
    rest = kcw - first
    out = [(0, first)]
    if rest:
        nb = -(-rest // 512)
        base = rest // nb
        base -= base % 32
        widths = [base] * (nb - 1)
        widths.append(rest - base * (nb - 1))
        off = first
        for w in widths:
            out.append((off, w))
            off += w
    return out


def build_nc(kc):
    kcw = kc * 128
    nc = bacc.Bacc(None, target_bir_lowering=False)

    m16 = nc.dram_tensor("m16", [128, HC, H], F16, kind="ExternalInput")
    hstc = nc.dram_tensor("hstc", [128, HC, kcw], F16, kind="ExternalInput")
    hst = nc.dram_tensor("hst", [128, HC, S], F16, kind="ExternalInput")
    hsb = nc.dram_tensor("hsb", [128, kc, H], F16, kind="ExternalInput")
    mkc = nc.dram_tensor("mkc", [128, kc], F32, kind="ExternalInput")
    out = nc.dram_tensor("out", [S, H], F32, kind="ExternalOutput")

    with tile.TileContext(nc) as tc, ExitStack() as whole:
        singles = whole.enter_context(tc.tile_pool(name="singles", bufs=1))
        res_pool = whole.enter_context(tc.tile_pool(name="res", bufs=1))

        junk = singles.tile([128, 512], F16, tag="junk", name="junk")
        nc.vector.memset(junk[:], 0.0)
        bias_sb = singles.tile([128, kc], F32, tag="bias", name="bias_sb")
        ones16 = singles.tile([128, 1], F16, tag="ones", name="ones16")
        nc.vector.memset(ones16[:], 1.0)

        # resident operands
        g16 = res_pool.tile([128, HC, kcw], F16, tag="g16", name="g16")
        hst_sb = res_pool.tile([128, HC, S], F16, tag="hst", name="hst_sb")
        hsb_sb = res_pool.tile([128, kc, H], F16, tag="hsb", name="hsb_sb")

        # PE warm-up: keep the PE ticking through the initial DMA wait.
        with tc.tile_pool(name="psw", bufs=1, space="PSUM") as psw:
            pjunk = psw.tile([128, 512], F32, tag="pj", name="pj")
            for _ in range(12):
                nc.tensor.matmul(
                    pjunk[:], lhsT=junk[:, 0:128], rhs=junk[:], start=True, stop=True
                )

        kbs = _kblocks(kcw)

        # ---- Phase A: fused projection g^T = M^T hs_c^T, fp16 into SBUF.
        # All input DMAs go on one queue, ordered by when the PE needs them:
        # m/hc block 0 first (first chain), then the remaining blocks, then
        # the phase-B/C residents. Serializing avoids bandwidth contention
        # on the startup-critical loads.
        with ExitStack() as pa:
            wt_pool = pa.enter_context(tc.tile_pool(name="wtp", bufs=1))
            hc_pool = pa.enter_context(tc.tile_pool(name="hcp", bufs=2))
            psA = pa.enter_context(tc.tile_pool(name="psA", bufs=4, space="PSUM"))

            # per-dout-chunk m tiles: the tile framework tracks dependencies
            # at tile granularity, so chain oc must only wait for its own
            # 0.25MB slice rather than the whole 2MB of M.
            m_sb = [
                wt_pool.tile([128, HC, 128], F16, tag=f"m{oc}", name=f"m{oc}")
                for oc in range(HC)
            ]
            # first m slice + first key block lead the sync queue (the fast
            # DMA engine) so the first chain's operands land right as the
            # PE warm-up ends. The gpsimd DMA engine is several times
            # slower for these strided patterns — only the tiny bias load
            # goes there.
            nc.sync.dma_start(out=m_sb[0][:], in_=m16.ap()[:, :, 0:128])
            hc0 = hc_pool.tile([128, HC, kbs[0][1]], F16, tag="hc0", name="hc0")
            nc.sync.dma_start(out=hc0[:], in_=hstc.ap()[:, :, 0:kbs[0][1]])
            nc.gpsimd.dma_start(out=bias_sb[:], in_=mkc.ap())
            for oc in range(1, HC):
                nc.sync.dma_start(
                    out=m_sb[oc][:],
                    in_=m16.ap()[:, :, oc * 128:(oc + 1) * 128],
                )

            for bi, (off, bw) in enumerate(kbs):
                if bi == 0:
                    hc = hc0
                else:
                    hc = hc_pool.tile([128, HC, bw], F16, tag="hc", name="hc")
                    nc.sync.dma_start(out=hc[:], in_=hstc.ap()[:, :, off:off + bw])
                if bi == len(kbs) - 1:
                    # phase-B/C residents: behind every phase-A load, ahead of
                    # their first use by ~30us of phase-A compute.
                    for qb in range(QB):
                        nc.sync.dma_start(
                            out=hst_sb[:, :, qb * 512:(qb + 1) * 512],
                            in_=hst.ap()[:, :, qb * 512:(qb + 1) * 512],
                        )
                    nc.sync.dma_start(out=hsb_sb[:], in_=hsb.ap())
                for oc in range(HC):
                    ps = psA.tile([128, 512], F32, tag="psA", name="psa")
                    for h in range(HC):
                        nc.tensor.matmul(
                            ps[:, 0:bw],
                            lhsT=m_sb[oc][:, h, :],
                            rhs=hc[:, h, :],
                            start=(h == 0),
                            stop=(h == HC - 1),
                        )
                    nc.scalar.copy(out=g16[:, oc, off:off + bw], in_=ps[:, 0:bw])

        # ---- Phase B: scoresT -> exp -> esum, per (key-chunk, query-block).
        with ExitStack() as pb:
            et_pool = pb.enter_context(tc.tile_pool(name="etp", bufs=1))
            es_pool = pb.enter_context(tc.tile_pool(name="esp", bufs=1))

            et = et_pool.tile([128, kc, S], F16, tag="et", name="et")
            esum = es_pool.tile([128, S], F16, tag="esum", name="esum")

            with tc.tile_pool(name="pss", bufs=3, space="PSUM") as ps_s:
                for c in range(kc):
                    for qb in range(QB):
                        ps = ps_s.tile([128, 512], F32, tag="pss", name="pss")
                        for d in range(HC):
                            nc.tensor.matmul(
                                ps[:],
                                lhsT=g16[:, d, c * 128:(c + 1) * 128],
                                rhs=hst_sb[:, d, qb * 512:(qb + 1) * 512],
                                start=(d == 0),
                                stop=(d == HC - 1),
                            )
                        sl = slice(qb * 512, (qb + 1) * 512)
                        nc.scalar.activation(
                            out=et[:, c, sl], in_=ps[:],
                            func=mybir.ActivationFunctionType.Exp,
                            scale=1.0 / 32.0,
                            bias=bias_sb[:, c:c + 1],
                        )
                        if c == 0:
                            nc.vector.tensor_scalar_mul(
                                out=esum[:, sl], in0=et[:, 0, sl], scalar1=1.0
                            )
                        else:
                            nc.vector.scalar_tensor_tensor(
                                out=esum[:, sl], in0=et[:, c, sl], scalar=1.0,
                                in1=esum[:, sl],
                                op0=mybir.AluOpType.mult, op1=mybir.AluOpType.add,
                            )

            # ---- Phase C: attention-value + normalization, per query tile.
            with ExitStack() as pc:
                ps_o = pc.enter_context(tc.tile_pool(name="pso", bufs=3, space="PSUM"))
                ps_n = pc.enter_context(tc.tile_pool(name="psn", bufs=2, space="PSUM"))
                out_pool = pc.enter_context(tc.tile_pool(name="outp", bufs=2))
                r_pool = pc.enter_context(tc.tile_pool(name="rp", bufs=4))

                for qs in range(QS):
                    qsl = slice(qs * 128, (qs + 1) * 128)
                    pn = ps_n.tile([128, 1], F32, tag="pn", name="pn")
                    nc.tensor.matmul(
                        pn[:], lhsT=esum[:, qsl], rhs=ones16[:], start=True, stop=True
                    )
                    r = r_pool.tile([128, 1], F32, tag="r", name="r")
                    nc.vector.reciprocal(r[:], pn[:, 0:1])
                    po0 = ps_o.tile([128, 512], F32, tag="po0", name="po0")
                    po1 = ps_o.tile([128, 512], F32, tag="po1", name="po1")
                    for c in range(kc):
                        st, sp = (c == 0), (c == kc - 1)
                        lw = et[:, c, qsl]
                        nc.tensor.matmul(po0[:], lhsT=lw, rhs=hsb_sb[:, c, 0:512],
                                         start=st, stop=sp)
                        nc.tensor.matmul(po1[:], lhsT=lw, rhs=hsb_sb[:, c, 512:1024],
                                         start=st, stop=sp)
                    ot = out_pool.tile([128, H], F32, tag="ot", name="ot")
                    rows = slice(qs * 128, (qs + 1) * 128)
                    nc.vector.tensor_scalar_mul(out=ot[:, 0:512], in0=po0[:], scalar1=r[:])
                    nc.scalar.dma_start(out=out.ap()[rows, 0:512], in_=ot[:, 0:512])
                    nc.vector.tensor_scalar_mul(out=ot[:, 512:1024], in0=po1[:], scalar1=r[:])
                    nc.scalar.dma_start(out=out.ap()[rows, 512:1024], in_=ot[:, 512:1024])

    nc.finalize()
    return nc


def _pack_T(x):
    """[S', H] -> hs^T arranged [128, HC, S'] (partition = h%128, chunk = h//128)."""
    return np.ascontiguousarray(
        x.T.reshape(HC, 128, x.shape[0]).transpose(1, 0, 2)
    )


def prepare(hidden_states, key_padding_mask, Wq_w, Wq_b, Wk_w, Wk_b):
    """Host-side folding + sharding. Returns (kc, in_maps)."""
    hs = np.ascontiguousarray(hidden_states, dtype=np.float32)
    mask = np.asarray(key_padding_mask, dtype=bool)
    wq = np.asarray(Wq_w, dtype=np.float64)
    wk = np.asarray(Wk_w, dtype=np.float64)
    bq = np.asarray(Wq_b, dtype=np.float64)
    m = (wk.T @ wq).astype(np.float32)                         # [h, d]
    u = (wk.T @ bq).astype(np.float32)                         # [h]

    counts = (~mask).sum(axis=1)
    kc = max(1, min(S // 128, int(-(-counts.max() // 128))))
    kcw = kc * 128

    m16 = np.ascontiguousarray(
        m.reshape(HC, 128, H).transpose(1, 0, 2).astype(np.float16)
    )

    in_maps = []
    for b in range(B):
        idx = np.nonzero(~mask[b])[0]
        hc = np.zeros((kcw, H), np.float32)
        hc[: len(idx)] = hs[b, idx]
        bias = np.full(kcw, -1e30, np.float32)
        bias[: len(idx)] = (hc[: len(idx)] @ u) / 32.0 - 3.0
        bias = np.ascontiguousarray(bias.reshape(kc, 128).T)   # [128, kc]
        h16 = hs[b].astype(np.float16)
        hc16 = hc.astype(np.float16)
        in_maps.append({
            "m16": m16,
            "hstc": _pack_T(hc16),
            "hst": _pack_T(h16),
            "hsb": np.ascontiguousarray(hc16.reshape(kc, 128, H).transpose(1, 0, 2)),
            "mkc": bias,
        })
    return kc, in_maps


def kernel(hidden_states, key_padding_mask, Wq_w, Wq_b, Wk_w, Wk_b):
    kc, in_maps = prepare(hidden_states, key_padding_mask, Wq_w, Wq_b, Wk_w, Wk_b)
    if kc not in _CACHED:
        _CACHED[kc] = build_nc(kc)
    nc = _CACHED[kc]
    res = run_bass_kernel_spmd(nc, in_maps, core_ids=list(range(N_CORES)))
    return np.stack([res.results[b]["out"] for b in range(B)]).astype(np.float32)
